# revision 5
# baseline (speedup 1.0000x reference)
"""MultiHeadCrossAttention Trainium2 Bass kernel (v2).

Sharding (8 cores): data-parallel over batch (2) x tensor-parallel over
head groups (4 groups of 4 heads).  Core c handles batch c//4, heads
4*(c%4) .. 4*(c%4)+3.  Each core computes TWO partial [Tq, D] outputs
(one per head-pair through its Wo row-slice); the host sums the 8
partials per batch.

Device math per core (all matmuls fp16 x fp16 -> fp32 PSUM):
  qT = Wq_s.T @ Xq.T          [256, Tq]   (head-dim on partitions)
  kT = Wk_s.T @ Xkv.T         [256, Tk]
  V  = Xkv @ Wv_s             [Tk, 256]   (+ ones column per head)
  St = kT_h.T @ qT_h          [Tk, Tq] scores^T, K=64, head pairs packed
                              into PE row-groups 0-63 / 64-127
  E  = exp(St/8)              (ScalarE, scale folded into activation)
  P  = E * expb               expb = exp(bias^T) * mask^T  (host-built;
                              multiplicative bias: exp(s+b) = exp(s)exp(b))
  [out^T; sums] = [V_h|1].T @ P   [65, Tq]  ones-column gives softmax sums
  out_norm^T = out^T * (1/sums)   (batched reciprocal, gpsimd broadcast+mul)
  partial[pair] = stack_pair^T.T @ Wo_pair  [Tq, D] per pair (host sums)

Scheduling: scores(t) and attnV(t-2) are interleaved on the PE so PSUM
needs only 2x psAB (4 banks) + 4x po accumulators (4 banks).  Wo is
pair-split and its tiles are interleaved into later units (or emitted
at the end for the last tq chunk) so the PE never waits on the
normalize tail mid-kernel.

Softmax max-subtraction is skipped: logits ~ N(0, ~1.1), max |logit| < ~7
over 16M samples, exp stays in fp16/fp32 range comfortably.
"""

from contextlib import ExitStack

import numpy as np

import concourse.bass as bass
import concourse.mybir as mybir
import concourse.tile as tile
from concourse import bacc
from concourse.bass_utils import run_bass_kernel_spmd

# Problem dims (hardcoded per contract).
D_MODEL = 1024
NUM_HEADS = 16
D_HEAD = 64
B = 2
TQ = 2048
TK = 2048
N_CORES = 8
HPC = 4  # heads per core
SCALE = 1.0 / 8.0  # 1/sqrt(D_HEAD)

F16 = mybir.dt.float16
F32 = mybir.dt.float32
NP_F16 = np.float16

NQ = 512  # matmul moving free-dim chunk (PSUM bank = 512 fp32)
CH = 1024  # scores tile free width (2 PSUM banks)


def build_nc(d_model=D_MODEL, tq=TQ, tk=TK, hpc=HPC, d_head=D_HEAD, scale=SCALE):
    """Build the single-core Bass program (SPMD: same NEFF on all cores)."""
    assert d_model % 128 == 0 and tq % CH == 0 and tk % 128 == 0
    assert hpc % 2 == 0
    ndt = d_model // 128          # contraction tiles for projections
    pairs = hpc // 2              # head pairs (128 head-dims per pair)
    hd = hpc * d_head             # per-core head dims (= 256)
    ntk = tk // 128               # Tk tiles of 128
    vw = d_head + 1               # V columns per head incl. ones column
    nqc = CH // NQ                # 512-chunks per scores tile (= 2)
    n_tqh = tq // CH              # tq macro-chunks (= 2)
    n_wot = CH // 128             # Wo row-tiles per tq chunk (= 8)

    nc = bacc.Bacc("TRN2", target_bir_lowering=False, debug=False)

    xq_d = nc.dram_tensor("xqT", [d_model, tq], F16, kind="ExternalInput")
    xkv_d = nc.dram_tensor("xkvT", [d_model, tk], F16, kind="ExternalInput")
    wq_d = nc.dram_tensor("wq", [d_model, hd], F16, kind="ExternalInput")
    wk_d = nc.dram_tensor("wk", [d_model, hd], F16, kind="ExternalInput")
    wv_d = nc.dram_tensor("wv", [d_model, hd], F16, kind="ExternalInput")
    wo_d = nc.dram_tensor("wo", [hd, d_model], F16, kind="ExternalInput")
    eb_d = nc.dram_tensor("expb", [hpc, tk, tq], F16, kind="ExternalInput")
    out_d = nc.dram_tensor("out", [pairs, tq, d_model], F16, kind="ExternalOutput")

    with ExitStack() as ctx:
        tc = ctx.enter_context(tile.TileContext(nc))
        # ---- persistent pools (SBUF)
        wpool = ctx.enter_context(tc.tile_pool(name="wpool", bufs=1))
        qkpool = ctx.enter_context(tc.tile_pool(name="qkpool", bufs=1))
        opool = ctx.enter_context(tc.tile_pool(name="opool", bufs=4))
        npool = ctx.enter_context(tc.tile_pool(name="npool", bufs=4))
        upool = ctx.enter_context(tc.tile_pool(name="upool", bufs=6))

        wq_sb = wpool.tile([128, ndt, hd], F16, tag="wq")
        wk_sb = wpool.tile([128, ndt, hd], F16, tag="wk")
        wv_sb = wpool.tile([128, ndt, hd], F16, tag="wv")
        wo_sb = wpool.tile([128, pairs, d_model], F16, tag="wo")

        qT_sb = qkpool.tile([128, pairs, tq], F16, tag="qT")
        kT_sb = qkpool.tile([128, pairs, tk], F16, tag="kT")
        v_sb = qkpool.tile([128, ntk, hpc * vw], F16, tag="v")
        stack_sb = qkpool.tile([128, pairs, tq], F16, tag="stack")

        # ones columns of v_sb (projection copies overwrite the V columns)
        nc.gpsimd.memset(v_sb[:], 1.0)

        # ---- phase A: projections (X^T resident only here)
        # psA: 4x [128,1024] fp32 slots (2 banks each) = all 8 PSUM banks;
        # closed before the phase-B psum pools open.
        with (
            tc.tile_pool(name="xpool", bufs=1) as xpool,
            tc.tile_pool(name="psA", bufs=4, space="PSUM") as psA,
        ):
            xkv_sb = [xpool.tile([128, tk], F16, tag=f"xkv{dt}", name="xkv_sb") for dt in range(ndt)]
            xq_sb = [xpool.tile([128, tq], F16, tag=f"xq{dt}", name="xq_sb") for dt in range(ndt)]
            # DMA order = DMA service order: wk, xkv (dt order), wq, xq, wv, wo
            nc.sync.dma_start(out=wk_sb[:], in_=wk_d.ap().rearrange("(t p) j -> p t j", p=128))
            for dt in range(ndt):
                nc.sync.dma_start(out=xkv_sb[dt][:], in_=xkv_d[dt * 128 : (dt + 1) * 128, :])
            nc.sync.dma_start(out=wq_sb[:], in_=wq_d.ap().rearrange("(t p) j -> p t j", p=128))
            for dt in range(ndt):
                nc.sync.dma_start(out=xq_sb[dt][:], in_=xq_d[dt * 128 : (dt + 1) * 128, :])
            nc.sync.dma_start(out=wv_sb[:], in_=wv_d.ap().rearrange("(t p) j -> p t j", p=128))
            nc.sync.dma_start(out=wo_sb[:], in_=wo_d.ap().rearrange("(t p) m -> p t m", p=128))

            # kT then qT: dt-outer so PE tracks the X DMA stream tile by
            # tile; (pair, half) accumulators fill all 4 psum slots.
            for wsb, xsb, dst, tlen in ((wk_sb, xkv_sb, kT_sb, tk), (wq_sb, xq_sb, qT_sb, tq)):
                psP = {
                    (j, c0): psA.tile([128, CH], F32, tag="psA", name="psP")
                    for j in range(pairs) for c0 in range(0, tlen, CH)
                }
                for dt in range(ndt):
                    for j in range(pairs):
                        for c0 in range(0, tlen, CH):
                            for q0 in range(0, CH, NQ):
                                nc.tensor.matmul(
                                    psP[j, c0][:, q0 : q0 + NQ],
                                    wsb[:, dt, j * 128 : (j + 1) * 128],
                                    xsb[dt][:, c0 + q0 : c0 + q0 + NQ],
                                    start=(dt == 0),
                                    stop=(dt == ndt - 1),
                                )
                for j in range(pairs):
                    for c0 in range(0, tlen, CH):
                        nc.scalar.copy(dst[:, j, c0 : c0 + CH], psP[j, c0][:])

            # V: [tk 128, hd] = X_kv @ Wv ; scatter per head next to ones cols
            for t in range(ntk):
                psv = psA.tile([128, hd], F32, tag="psA", name="psv")
                for dt in range(ndt):
                    nc.tensor.matmul(
                        psv[:],
                        xkv_sb[dt][:, t * 128 : (t + 1) * 128],
                        wv_sb[:, dt, :],
                        start=(dt == 0),
                        stop=(dt == ndt - 1),
                    )
                nc.scalar.copy(
                    v_sb[:, t, :].rearrange("p (h w) -> p h w", w=vw)[:, :, 0:d_head],
                    psv[:].rearrange("p (h w) -> p h w", w=d_head),
                )

        # ---- phase B: attention units (tqh, pair), scores/attnV interleaved.
        # Wo for tq chunk tqh is emitted inside the units of chunk tqh+1
        # (pair-split: each pair's Wo gated only on its own tail), except the
        # last chunk whose Wo runs at the end.
        with (
            tc.tile_pool(name="ppool", bufs=8) as ppool,
            tc.tile_pool(name="ebpool", bufs=12) as ebpool,
            tc.tile_pool(name="psS", bufs=2, space="PSUM") as psS,
            tc.tile_pool(name="psO", bufs=4, space="PSUM") as psO,
        ):
            # pending Wo work: list of (tqh, pair, ti)
            wo_queue = []

            def emit_wo(n):
                for _ in range(n):
                    if not wo_queue:
                        return
                    tqh, pair, ti = wo_queue.pop(0)
                    t = tqh * n_wot + ti
                    pf = psS.tile([128, d_model], F32, tag="ps", name="pf")
                    for m0 in range(0, d_model, NQ):
                        nc.tensor.matmul(
                            pf[:, m0 : m0 + NQ],
                            stack_sb[:, pair, t * 128 : (t + 1) * 128],
                            wo_sb[:, pair, m0 : m0 + NQ],
                            start=True,
                            stop=True,
                        )
                    osb = opool.tile([128, d_model], F16, tag="osb", name="osb")
                    eng = nc.vector.tensor_copy if ti % 2 == 0 else nc.scalar.copy
                    eng(osb[:], pf[:])
                    nc.sync.dma_start(out=out_d[pair, t * 128 : (t + 1) * 128, :], in_=osb[:])

            for tqh in range(n_tqh):
                c0 = tqh * CH
                for pair in range(pairs):
                    po = [[None] * nqc for _ in range(2)]
                    p_ts = [None] * ntk
                    eb_ts = [None] * ntk

                    def attn_v(t):
                        for hh in range(2):
                            h = 2 * pair + hh
                            for qi in range(nqc):
                                if t == 0:
                                    po[hh][qi] = psO.tile([vw, NQ], F32, tag="po", name="po")
                                nc.tensor.matmul(
                                    po[hh][qi][:],
                                    v_sb[:, t, h * vw : (h + 1) * vw],
                                    p_ts[t][hh][:, qi * NQ : (qi + 1) * NQ],
                                    start=(t == 0),
                                    stop=(t == ntk - 1),
                                )
                        p_ts[t] = None  # release p tile

                    for t in range(ntk):
                        tr = slice(t * 128, (t + 1) * 128)
                        eb_t = ebpool.tile([128, 2, CH], F16, tag="eb", name="eb")
                        nc.sync.dma_start(
                            out=eb_t[:],
                            in_=eb_d.ap()[2 * pair : 2 * pair + 2, tr,
                                          c0 : c0 + CH].rearrange("h p q -> p h q"),
                        )
                        eb_ts[t] = eb_t
                        # scores(t): psAB per head, exp on ACT, expb-mul on DVE
                        pp = []
                        for hh in range(2):
                            r0 = hh * 64
                            psAB = psS.tile([128, CH], F32, tag="ps", name="ps")
                            for q0 in range(0, CH, NQ):
                                nc.tensor.matmul(
                                    psAB[:, q0 : q0 + NQ],
                                    kT_sb[r0 : r0 + 64, pair, tr],
                                    qT_sb[r0 : r0 + 64, pair, c0 + q0 : c0 + q0 + NQ],
                                    start=True,
                                    stop=True,
                                )
                            p_t = ppool.tile([128, CH], F16, tag="p", name="p_t")
                            nc.scalar.activation(
                                out=p_t[:], in_=psAB[:],
                                func=mybir.ActivationFunctionType.Exp, scale=scale,
                            )
                            nc.vector.tensor_mul(p_t[:], p_t[:], eb_t[:, hh, :])
                            pp.append(p_t)
                        p_ts[t] = pp
                        if t >= 2:
                            attn_v(t - 2)
                            if t % 4 == 1:
                                emit_wo(1)
                    attn_v(ntk - 2)
                    attn_v(ntk - 1)

                    # ---- normalize tail for this unit
                    sums_t = npool.tile([2 * nqc, NQ], F32, tag="sums", name="sums_t", bufs=2)
                    u_list = []
                    for hh in range(2):
                        for qi in range(nqc):
                            row = hh * nqc + qi
                            qg = tqh * nqc + qi
                            u_t = upool.tile([64, NQ], F16, tag="u", name="u_t")
                            nc.vector.tensor_copy(u_t[:], po[hh][qi][0:64, :])
                            sm_t = npool.tile([1, NQ], F32, tag="sm", name="sm_t")
                            nc.vector.tensor_copy(sm_t[:], po[hh][qi][64:65, :])
                            nc.sync.dma_start(out=sums_t[row : row + 1, :], in_=sm_t[:])
                            u_list.append((u_t, row, hh * 64, qg))
                    recip_f = npool.tile([2 * nqc, NQ], F32, tag="recipf", name="recip_f", bufs=2)
                    nc.vector.reciprocal(out=recip_f[:], in_=sums_t[:])
                    recip16 = npool.tile([2 * nqc, NQ], F16, tag="recip", name="recip16", bufs=2)
                    nc.vector.tensor_copy(recip16[:], recip_f[:])
                    for u_t, row, r0, qg in u_list:
                        r_t = npool.tile([1, NQ], F16, tag="r", name="r_t")
                        nc.sync.dma_start(out=r_t[:], in_=recip16[row : row + 1, :])
                        rb_t = npool.tile([64, NQ], F16, tag="rb", name="rb_t")
                        nc.gpsimd.partition_broadcast(rb_t[:], r_t[:])
                        nc.gpsimd.tensor_mul(
                            stack_sb[r0 : r0 + 64, pair, qg * NQ : (qg + 1) * NQ],
                            u_t[:],
                            rb_t[:],
                        )
                    # this unit's Wo becomes available for later interleave
                    wo_queue.extend((tqh, pair, ti) for ti in range(n_wot))
                # boundary between tq chunks: drain half the backlog
                emit_wo(6)
            emit_wo(len(wo_queue))

    nc.compile()
    return nc


_NC = None
LAST_RESULTS = None


def _get_nc():
    global _NC
    if _NC is None:
        _NC = build_nc()
    return _NC


def _shard_inputs(query, key_value, mask, rel_pos_bias, Wq, Wkv, Wo):
    """Build the 8 per-core input maps (host-side transposes + exp-bias)."""
    in_maps = []
    w_f16 = {
        "Wq": Wq.astype(NP_F16),
        "Wo": Wo.astype(NP_F16),
        "Wkv": Wkv.astype(NP_F16),
    }
    for c in range(N_CORES):
        b = c // (N_CORES // B)
        g = c % (N_CORES // B)
        cs = slice(g * HPC * D_HEAD, (g + 1) * HPC * D_HEAD)
        hs = slice(g * HPC, (g + 1) * HPC)
        # expb = exp(bias)^T * mask^T   (fp32 exp, fp16 ship)
        eb = np.exp(rel_pos_bias[hs].astype(np.float32)).transpose(0, 2, 1)
        eb = eb * mask[b, 0].T[None].astype(np.float32)
        in_maps.append({
            "xqT": np.ascontiguousarray(query[b].T).astype(NP_F16),
            "xkvT": np.ascontiguousarray(key_value[b].T).astype(NP_F16),
            "wq": w_f16["Wq"][:, cs].copy(),
            "wk": w_f16["Wkv"][:, cs].copy(),
            "wv": w_f16["Wkv"][:, D_MODEL + cs.start : D_MODEL + cs.stop].copy(),
            "wo": w_f16["Wo"][cs, :].copy(),
            "expb": eb.astype(NP_F16),
        })
    return in_maps


def kernel(query, key_value, mask, rel_pos_bias, Wq, Wkv, Wo):
    global LAST_RESULTS
    query, key_value, mask, rel_pos_bias, Wq, Wkv, Wo = (
        np.asarray(a) for a in (query, key_value, mask, rel_pos_bias, Wq, Wkv, Wo)
    )
    nc = _get_nc()
    in_maps = _shard_inputs(query, key_value, mask, rel_pos_bias, Wq, Wkv, Wo)
    res = run_bass_kernel_spmd(nc, in_maps, core_ids=list(range(N_CORES)))
    LAST_RESULTS = res
    gpc = N_CORES // B  # cores per batch group
    out = np.stack([
        sum(res.results[b * gpc + i]["out"].astype(np.float32).sum(axis=0) for i in range(gpc))
        for b in range(B)
    ])
    return out


# revision 8
# speedup vs baseline: 1.1446x; 1.1446x over previous
"""MultiHeadCrossAttention Trainium2 Bass kernel (v3).

Sharding (8 cores): data-parallel over batch (2) x tensor-parallel over
head groups (4 groups of 4 heads).  Core c handles batch c//4, heads
4*(c%4) .. 4*(c%4)+3.  Each core computes TWO partial [Tq, D] outputs
(one per head-pair through its Wo row-slice); the host sums the 8
partials per batch.

Device math per core (all matmuls fp16 x fp16 -> fp32 PSUM):
  qT = Wq_s.T @ Xq.T          [256, Tq]   (head-dim on partitions)
  kT = Wk_s.T @ Xkv.T         [256, Tk]
  V  = Xkv @ Wv_s             [Tk, 256]   (+ ones column per head)
  St = kT_h.T @ qT_h          [Tk, Tq] scores^T, K=64, head pairs packed
                              into PE row-groups 0-63 / 64-127
  E  = exp(St/8)              (ScalarE, scale folded into activation)
  P  = E * expb               expb = exp(bias^T) * mask^T  (host-built;
                              multiplicative bias: exp(s+b) = exp(s)exp(b))
  [out^T; sums] = [V_h|1].T @ P   [65, Tq]  ones-column gives softmax sums
  out_norm^T = out^T * (1/sums)   (approx reciprocal + gpsimd broadcast)
  partial[pair] = stack_pair.T @ Wo_pair  [Tq, D] per pair (host sums)

Wo is pair-split and emitted at tq-chunk boundaries ordered pair0 then
pair1, so pair1's normalize tail hides under pair0's Wo matmuls.  The
small tail DMAs (sums gather, reciprocal row moves) ride the Vector
engine's DGE ring so they never queue behind the bulk expb prefetch
stream on the Sync ring.

Softmax max-subtraction is skipped: logits ~ N(0, ~1.1), max |logit| < ~7
over 16M samples, exp stays in fp16/fp32 range comfortably.
"""

import os
from contextlib import ExitStack

import numpy as np

import concourse.bass as bass
import concourse.mybir as mybir
import concourse.tile as tile
from concourse import bacc
from concourse.bass_utils import run_bass_kernel_spmd

# Problem dims (hardcoded per contract).
D_MODEL = 1024
NUM_HEADS = 16
D_HEAD = 64
B = 2
TQ = 2048
TK = 2048
N_CORES = 8
HPC = 4  # heads per core
SCALE = 1.0 / 8.0  # 1/sqrt(D_HEAD)

F16 = mybir.dt.float16
F32 = mybir.dt.float32
NP_F16 = np.float16

NQ = 512  # matmul moving free-dim chunk (PSUM bank = 512 fp32)


def build_nc(d_model=D_MODEL, tq=TQ, tk=TK, hpc=HPC, d_head=D_HEAD, scale=SCALE):
    """Build the single-core Bass program (SPMD: same NEFF on all cores)."""
    assert d_model % 128 == 0 and tq % NQ == 0 and tk % 128 == 0
    assert hpc % 2 == 0
    ndt = d_model // 128          # contraction tiles for projections
    pairs = hpc // 2              # head pairs (128 head-dims per pair)
    hd = hpc * d_head             # per-core head dims (= 256)
    ntq = tq // NQ                # Tq chunks of 512
    ntk = tk // 128               # Tk tiles of 128
    vw = d_head + 1               # V columns per head incl. ones column
    CH = min(tq, 1024)            # scores psum tile width (2 PSUM banks)
    nqc = CH // NQ                # 512-chunks per scores tile
    n_tqh = tq // CH              # tq macro-chunks per head
    n_wot = CH // 128             # Wo row-tiles per tq chunk (= 8)

    nc = bacc.Bacc("TRN2", target_bir_lowering=False, debug=False)

    xq_d = nc.dram_tensor("xqT", [d_model, tq], F16, kind="ExternalInput")
    xkv_d = nc.dram_tensor("xkvT", [d_model, tk], F16, kind="ExternalInput")
    wq_d = nc.dram_tensor("wq", [d_model, hd], F16, kind="ExternalInput")
    wk_d = nc.dram_tensor("wk", [d_model, hd], F16, kind="ExternalInput")
    wv_d = nc.dram_tensor("wv", [d_model, hd], F16, kind="ExternalInput")
    wo_d = nc.dram_tensor("wo", [hd, d_model], F16, kind="ExternalInput")
    eb_d = nc.dram_tensor("expb", [hpc, tk, tq], F16, kind="ExternalInput")
    out_d = nc.dram_tensor("out", [pairs, tq, d_model], F16, kind="ExternalOutput")

    with ExitStack() as ctx:
        tc = ctx.enter_context(tile.TileContext(nc))
        # ---- persistent pools
        wpool = ctx.enter_context(tc.tile_pool(name="wpool", bufs=1))
        qkpool = ctx.enter_context(tc.tile_pool(name="qkpool", bufs=1))
        opool = ctx.enter_context(tc.tile_pool(name="opool", bufs=3))
        npool = ctx.enter_context(tc.tile_pool(name="npool", bufs=4))
        upool = ctx.enter_context(tc.tile_pool(name="upool", bufs=2 * nqc * 2))
        psS = ctx.enter_context(tc.tile_pool(name="psS", bufs=2, space="PSUM"))
        psO = ctx.enter_context(tc.tile_pool(name="psO", bufs=4, space="PSUM"))

        wq_sb = wpool.tile([128, ndt, hd], F16, tag="wq")
        wk_sb = wpool.tile([128, ndt, hd], F16, tag="wk")
        wv_sb = wpool.tile([128, ndt, hd], F16, tag="wv")
        wo_sb = wpool.tile([128, pairs, d_model], F16, tag="wo")
        nc.sync.dma_start(out=wk_sb[:], in_=wk_d.ap().rearrange("(t p) j -> p t j", p=128))
        nc.sync.dma_start(out=wv_sb[:], in_=wv_d.ap().rearrange("(t p) j -> p t j", p=128))

        qT_sb = qkpool.tile([128, pairs, tq], F16, tag="qT")
        kT_sb = qkpool.tile([128, pairs, tk], F16, tag="kT")
        v_sb = qkpool.tile([128, ntk, hpc * vw], F16, tag="v")
        stack_sb = qkpool.tile([128, pairs, tq], F16, tag="stack")

        # ones columns of v_sb (projection copies overwrite the V columns)
        nc.gpsimd.memset(v_sb[:], 1.0)

        # ---- phase A: projections (X^T resident only here)
        with tc.tile_pool(name="xpool", bufs=1) as xpool:
            # one tile per d-slice so each projection matmul depends only on
            # its own 0.5 MB DMA (kv first: kT, V and scores need it)
            xkv_sb = [xpool.tile([128, tk], F16, tag=f"xkv{dt}", name="xkv_sb") for dt in range(ndt)]
            xq_sb = [xpool.tile([128, tq], F16, tag=f"xq{dt}", name="xq_sb") for dt in range(ndt)]
            for dt in range(ndt):
                nc.sync.dma_start(out=xkv_sb[dt][:], in_=xkv_d[dt * 128 : (dt + 1) * 128, :])
            nc.sync.dma_start(out=wq_sb[:], in_=wq_d.ap().rearrange("(t p) j -> p t j", p=128))
            for dt in range(ndt):
                nc.sync.dma_start(out=xq_sb[dt][:], in_=xq_d[dt * 128 : (dt + 1) * 128, :])
            nc.sync.dma_start(out=wo_sb[:], in_=wo_d.ap().rearrange("(t p) m -> p t m", p=128))

            # qT / kT: [j-pair 128, tq]  = sum_d W[:, j].T @ X^T
            for wsb, xsb, dst, tlen in ((wk_sb, xkv_sb, kT_sb, tk), (wq_sb, xq_sb, qT_sb, tq)):
                for j in range(pairs):
                    for c0 in range(0, tlen, CH):
                        cn = min(CH, tlen - c0)
                        ps = psS.tile([128, cn], F32, tag="ps", name="ps")
                        for dt in range(ndt):
                            for q0 in range(0, cn, NQ):
                                qn = min(NQ, cn - q0)
                                nc.tensor.matmul(
                                    ps[:, q0 : q0 + qn],
                                    wsb[:, dt, j * 128 : (j + 1) * 128],
                                    xsb[dt][:, c0 + q0 : c0 + q0 + qn],
                                    start=(dt == 0),
                                    stop=(dt == ndt - 1),
                                )
                        nc.vector.tensor_copy(dst[:, j, c0 : c0 + cn], ps[:])

            # V: [tk 128, hd] = X_kv @ Wv ; scatter per head next to ones cols
            for t in range(ntk):
                psv = psO.tile([128, hd], F32, tag="po", name="psv")
                for dt in range(ndt):
                    nc.tensor.matmul(
                        psv[:],
                        xkv_sb[dt][:, t * 128 : (t + 1) * 128],
                        wv_sb[:, dt, :],
                        start=(dt == 0),
                        stop=(dt == ndt - 1),
                    )
                nc.vector.tensor_copy(
                    v_sb[:, t, :].rearrange("p (h w) -> p h w", w=vw)[:, :, 0:d_head],
                    psv[:].rearrange("p (h w) -> p h w", w=d_head),
                )

        # ---- phase B: attention, phased per (tqh, pair) unit:
        # scores+exp+mul batch, then attnV batch, then the normalize tail.
        # Wo (pair-split) runs at tq-chunk boundaries: pair0 first (its tail
        # finished during pair1's unit), then pair1 (tail hides under pair0's
        # Wo matmuls).
        with (
            tc.tile_pool(name="ppool", bufs=2 * ntk + 12) as ppool,
            tc.tile_pool(name="ebpool", bufs=6) as ebpool,
        ):
            def emit_wo(tqh, pair):
                for ti in range(n_wot):
                    t = tqh * n_wot + ti
                    pf = psS.tile([128, d_model], F32, tag="ps", name="pf")
                    for m0 in range(0, d_model, NQ):
                        nc.tensor.matmul(
                            pf[:, m0 : m0 + NQ],
                            stack_sb[:, pair, t * 128 : (t + 1) * 128],
                            wo_sb[:, pair, m0 : m0 + NQ],
                            start=True,
                            stop=True,
                        )
                    osb = opool.tile([128, d_model], F16, tag="osb", name="osb")
                    eng = nc.vector.tensor_copy if ti % 2 == 0 else nc.scalar.copy
                    eng(osb[:], pf[:])
                    nc.sync.dma_start(out=out_d[pair, t * 128 : (t + 1) * 128, :], in_=osb[:])

            for tqh in range(n_tqh):
                c0 = tqh * CH
                for pair in range(pairs):
                    # scores^T + exp + expb-mul for both heads of the pair
                    p_ts = []
                    for t in range(ntk):
                        tr = slice(t * 128, (t + 1) * 128)
                        eb_t = ebpool.tile([128, 2, CH], F16, tag="eb", name="eb")
                        nc.sync.dma_start(
                            out=eb_t[:],
                            in_=eb_d.ap()[2 * pair : 2 * pair + 2, t * 128 : (t + 1) * 128,
                                          c0 : c0 + CH].rearrange("h p q -> p h q"),
                        )
                        pp = []
                        for hh in range(2):
                            r0 = hh * 64
                            psAB = psS.tile([128, CH], F32, tag="ps", name="ps")
                            for q0 in range(0, CH, NQ):
                                nc.tensor.matmul(
                                    psAB[:, q0 : q0 + NQ],
                                    kT_sb[r0 : r0 + 64, pair, tr],
                                    qT_sb[r0 : r0 + 64, pair, c0 + q0 : c0 + q0 + NQ],
                                    start=True,
                                    stop=True,
                                )
                            p_t = ppool.tile([128, CH], F16, tag="p", name="p_t")
                            nc.scalar.activation(
                                out=p_t[:], in_=psAB[:],
                                func=mybir.ActivationFunctionType.Exp, scale=scale,
                            )
                            nc.vector.tensor_mul(p_t[:], p_t[:], eb_t[:, hh, :])
                            pp.append(p_t)
                        p_ts.append(pp)

                    # attn @ [V|1] -> [65, NQ] per (head, 512-chunk)
                    sums_t = npool.tile([2 * nqc, NQ], F32, tag="sums", name="sums_t", bufs=2)
                    u_list = []
                    for hh in range(2):
                        h = 2 * pair + hh
                        po = [psO.tile([vw, NQ], F32, tag="po", name="po") for _ in range(nqc)]
                        for t in range(ntk):
                            for qi in range(nqc):
                                nc.tensor.matmul(
                                    po[qi][:],
                                    v_sb[:, t, h * vw : (h + 1) * vw],
                                    p_ts[t][hh][:, qi * NQ : (qi + 1) * NQ],
                                    start=(t == 0),
                                    stop=(t == ntk - 1),
                                )
                        for qi in range(nqc):
                            qg = tqh * nqc + qi  # global 512-chunk index
                            row = hh * nqc + qi
                            u_t = upool.tile([64, NQ], F16, tag="u", name="u_t")
                            nc.vector.tensor_copy(u_t[:], po[qi][0:64, :])
                            sm_t = npool.tile([1, NQ], F32, tag="sm", name="sm_t")
                            nc.vector.tensor_copy(sm_t[:], po[qi][64:65, :])
                            # gather row via DMA on the Activation ring (keeps
                            # clear of the bulk expb stream on the Sync ring)
                            nc.scalar.dma_start(out=sums_t[row : row + 1, :], in_=sm_t[:])
                            u_list.append((u_t, row, hh * 64, qg))

                    # normalize: one fast reciprocal per unit, gpsimd
                    # broadcast straight from the recip row, multiply on DVE
                    recip_f = npool.tile([2 * nqc, NQ], F32, tag="recipf", name="recip_f", bufs=2)
                    nc.vector.reciprocal_approx_fast(out=recip_f[:], in_=sums_t[:])
                    recip16 = npool.tile([2 * nqc, NQ], F16, tag="recip", name="recip16", bufs=2)
                    nc.vector.tensor_copy(recip16[:], recip_f[:])
                    for u_t, row, r0, qg in u_list:
                        r_t = npool.tile([1, NQ], F16, tag="r", name="r_t")
                        nc.scalar.dma_start(out=r_t[:], in_=recip16[row : row + 1, :])
                        rb_t = npool.tile([64, NQ], F16, tag="rb", name="rb_t")
                        nc.gpsimd.partition_broadcast(rb_t[:], r_t[:])
                        nc.vector.tensor_mul(
                            stack_sb[r0 : r0 + 64, pair, qg * NQ : (qg + 1) * NQ],
                            u_t[:],
                            rb_t[:],
                        )
                # boundary: out-projection for this tq chunk, pair0 then pair1
                for pair in range(pairs):
                    emit_wo(tqh, pair)

    nc.compile()
    return nc


_NC = None
LAST_RESULTS = None


def _get_nc():
    global _NC
    if _NC is None:
        _NC = build_nc()
    return _NC


def _shard_inputs(query, key_value, mask, rel_pos_bias, Wq, Wkv, Wo):
    """Build the 8 per-core input maps (host-side transposes + exp-bias)."""
    in_maps = []
    w_f16 = {
        "Wq": Wq.astype(NP_F16),
        "Wo": Wo.astype(NP_F16),
        "Wkv": Wkv.astype(NP_F16),
    }
    for c in range(N_CORES):
        b = c // (N_CORES // B)
        g = c % (N_CORES // B)
        cs = slice(g * HPC * D_HEAD, (g + 1) * HPC * D_HEAD)
        hs = slice(g * HPC, (g + 1) * HPC)
        # expb = exp(bias)^T * mask^T   (fp32 exp, fp16 ship)
        eb = np.exp(rel_pos_bias[hs].astype(np.float32)).transpose(0, 2, 1)
        eb = eb * mask[b, 0].T[None].astype(np.float32)
        in_maps.append({
            "xqT": np.ascontiguousarray(query[b].T).astype(NP_F16),
            "xkvT": np.ascontiguousarray(key_value[b].T).astype(NP_F16),
            "wq": w_f16["Wq"][:, cs].copy(),
            "wk": w_f16["Wkv"][:, cs].copy(),
            "wv": w_f16["Wkv"][:, D_MODEL + cs.start : D_MODEL + cs.stop].copy(),
            "wo": w_f16["Wo"][cs, :].copy(),
            "expb": eb.astype(NP_F16),
        })
    return in_maps


def kernel(query, key_value, mask, rel_pos_bias, Wq, Wkv, Wo):
    global LAST_RESULTS
    query, key_value, mask, rel_pos_bias, Wq, Wkv, Wo = (
        np.asarray(a) for a in (query, key_value, mask, rel_pos_bias, Wq, Wkv, Wo)
    )
    nc = _get_nc()
    in_maps = _shard_inputs(query, key_value, mask, rel_pos_bias, Wq, Wkv, Wo)
    res = run_bass_kernel_spmd(nc, in_maps, core_ids=list(range(N_CORES)))
    LAST_RESULTS = res
    gpc = N_CORES // B  # cores per batch group
    out = np.stack([
        sum(res.results[b * gpc + i]["out"].astype(np.float32).sum(axis=0) for i in range(gpc))
        for b in range(B)
    ])
    return out


# revision 9
# speedup vs baseline: 1.6236x; 1.4185x over previous
"""MultiHeadCrossAttention Trainium2 Bass kernel (v3).

Sharding (8 cores): data-parallel over batch (2) x tensor-parallel over
head groups (4 groups of 4 heads).  Core c handles batch c//4, heads
4*(c%4) .. 4*(c%4)+3.  Each core computes TWO partial [Tq, D] outputs
(one per head-pair through its Wo row-slice); the host sums the 8
partials per batch.

Device math per core (all matmuls fp16 x fp16 -> fp32 PSUM):
  qT = Wq_s.T @ Xq.T          [256, Tq]   (head-dim on partitions)
  kT = Wk_s.T @ Xkv.T         [256, Tk]
  V  = Xkv @ Wv_s             [Tk, 256]   (+ ones column per head)
  St = kT_h.T @ qT_h          [Tk, Tq] scores^T, K=64, head pairs packed
                              into PE row-groups 0-63 / 64-127
  E  = exp(St/8)              (ScalarE, scale folded into activation)
  P  = E * expb               expb = exp(bias^T) * mask^T  (host-built;
                              multiplicative bias: exp(s+b) = exp(s)exp(b))
  [out^T; sums] = [V_h|1].T @ P   [65, Tq]  ones-column gives softmax sums
  out_norm^T = out^T * (1/sums)   (approx reciprocal + gpsimd broadcast)
  partial[pair] = stack_pair.T @ Wo_pair  [Tq, D] per pair (host sums)

Wo is pair-split and emitted at tq-chunk boundaries ordered pair0 then
pair1, so pair1's normalize tail hides under pair0's Wo matmuls.  The
small tail DMAs (sums gather, reciprocal row moves) ride the Vector
engine's DGE ring so they never queue behind the bulk expb prefetch
stream on the Sync ring.

Softmax max-subtraction is skipped: logits ~ N(0, ~1.1), max |logit| < ~7
over 16M samples, exp stays in fp16/fp32 range comfortably.
"""

import os
from contextlib import ExitStack

import numpy as np

import concourse.bass as bass
import concourse.mybir as mybir
import concourse.tile as tile
from concourse import bacc
from concourse.bass_utils import run_bass_kernel_spmd

# Problem dims (hardcoded per contract).
D_MODEL = 1024
NUM_HEADS = 16
D_HEAD = 64
B = 2
TQ = 2048
TK = 2048
N_CORES = 8
HPC = 4  # heads per core
SCALE = 1.0 / 8.0  # 1/sqrt(D_HEAD)

F16 = mybir.dt.float16
F32 = mybir.dt.float32
NP_F16 = np.float16

NQ = 512  # matmul moving free-dim chunk (PSUM bank = 512 fp32)


def build_nc(d_model=D_MODEL, tq=TQ, tk=TK, hpc=HPC, d_head=D_HEAD, scale=SCALE):
    """Build the single-core Bass program (SPMD: same NEFF on all cores)."""
    assert d_model % 128 == 0 and tq % NQ == 0 and tk % 128 == 0
    assert hpc % 2 == 0
    ndt = d_model // 128          # contraction tiles for projections
    pairs = hpc // 2              # head pairs (128 head-dims per pair)
    hd = hpc * d_head             # per-core head dims (= 256)
    ntq = tq // NQ                # Tq chunks of 512
    ntk = tk // 128               # Tk tiles of 128
    vw = d_head + 1               # V columns per head incl. ones column
    CH = min(tq, 1024)            # scores psum tile width (2 PSUM banks)
    nqc = CH // NQ                # 512-chunks per scores tile
    n_tqh = tq // CH              # tq macro-chunks per head
    n_wot = CH // 128             # Wo row-tiles per tq chunk (= 8)

    nc = bacc.Bacc("TRN2", target_bir_lowering=False, debug=False)

    xq_d = nc.dram_tensor("xqT", [d_model, tq], F16, kind="ExternalInput")
    xkv_d = nc.dram_tensor("xkvT", [d_model, tk], F16, kind="ExternalInput")
    wq_d = nc.dram_tensor("wq", [d_model, hd], F16, kind="ExternalInput")
    wk_d = nc.dram_tensor("wk", [d_model, hd], F16, kind="ExternalInput")
    wv_d = nc.dram_tensor("wv", [d_model, hd], F16, kind="ExternalInput")
    wo_d = nc.dram_tensor("wo", [hd, d_model], F16, kind="ExternalInput")
    eb_d = nc.dram_tensor("expb", [hpc, tk, tq], F16, kind="ExternalInput")
    out_d = nc.dram_tensor("out", [pairs, tq, d_model], F16, kind="ExternalOutput")

    with ExitStack() as ctx:
        tc = ctx.enter_context(tile.TileContext(nc))
        # ---- persistent pools
        wpool = ctx.enter_context(tc.tile_pool(name="wpool", bufs=1))
        qkpool = ctx.enter_context(tc.tile_pool(name="qkpool", bufs=1))
        opool = ctx.enter_context(tc.tile_pool(name="opool", bufs=3))
        npool = ctx.enter_context(tc.tile_pool(name="npool", bufs=4))
        upool = ctx.enter_context(tc.tile_pool(name="upool", bufs=hpc * (tq // NQ)))
        psS = ctx.enter_context(tc.tile_pool(name="psS", bufs=3, space="PSUM"))
        psO = ctx.enter_context(tc.tile_pool(name="psO", bufs=2, space="PSUM"))

        wq_sb = wpool.tile([128, ndt, hd], F16, tag="wq")
        wk_sb = wpool.tile([128, ndt, hd], F16, tag="wk")
        wv_sb = wpool.tile([128, ndt, hd], F16, tag="wv")
        wo_sb = wpool.tile([128, pairs, d_model], F16, tag="wo")
        nc.sync.dma_start(out=wk_sb[:], in_=wk_d.ap().rearrange("(t p) j -> p t j", p=128))
        nc.sync.dma_start(out=wv_sb[:], in_=wv_d.ap().rearrange("(t p) j -> p t j", p=128))

        qT_sb = qkpool.tile([128, pairs, tq], F16, tag="qT")
        kT_sb = qkpool.tile([128, pairs, tk], F16, tag="kT")
        v_sb = qkpool.tile([128, ntk, hpc * vw], F16, tag="v")
        stack_sb = qkpool.tile([128, pairs, tq], F16, tag="stack")

        # ones columns of v_sb (projection copies overwrite the V columns)
        nc.gpsimd.memset(v_sb[:], 1.0)

        # ---- phase A: projections (X^T resident only here)
        with tc.tile_pool(name="xpool", bufs=1) as xpool:
            # one tile per d-slice so each projection matmul depends only on
            # its own 0.5 MB DMA (kv first: kT, V and scores need it)
            xkv_sb = [xpool.tile([128, tk], F16, tag=f"xkv{dt}", name="xkv_sb") for dt in range(ndt)]
            xq_sb = [xpool.tile([128, tq], F16, tag=f"xq{dt}", name="xq_sb") for dt in range(ndt)]
            for dt in range(ndt):
                nc.sync.dma_start(out=xkv_sb[dt][:], in_=xkv_d[dt * 128 : (dt + 1) * 128, :])
            nc.sync.dma_start(out=wq_sb[:], in_=wq_d.ap().rearrange("(t p) j -> p t j", p=128))
            for dt in range(ndt):
                nc.sync.dma_start(out=xq_sb[dt][:], in_=xq_d[dt * 128 : (dt + 1) * 128, :])
            nc.sync.dma_start(out=wo_sb[:], in_=wo_d.ap().rearrange("(t p) m -> p t m", p=128))

            # qT / kT: [j-pair 128, tq]  = sum_d W[:, j].T @ X^T
            for wsb, xsb, dst, tlen in ((wk_sb, xkv_sb, kT_sb, tk), (wq_sb, xq_sb, qT_sb, tq)):
                for j in range(pairs):
                    for c0 in range(0, tlen, CH):
                        cn = min(CH, tlen - c0)
                        ps = psS.tile([128, cn], F32, tag="ps", name="ps")
                        for dt in range(ndt):
                            for q0 in range(0, cn, NQ):
                                qn = min(NQ, cn - q0)
                                nc.tensor.matmul(
                                    ps[:, q0 : q0 + qn],
                                    wsb[:, dt, j * 128 : (j + 1) * 128],
                                    xsb[dt][:, c0 + q0 : c0 + q0 + qn],
                                    start=(dt == 0),
                                    stop=(dt == ndt - 1),
                                )
                        nc.vector.tensor_copy(dst[:, j, c0 : c0 + cn], ps[:])

            # V: [tk 128, hd] = X_kv @ Wv ; scatter per head next to ones cols
            for t in range(ntk):
                psv = psO.tile([128, hd], F32, tag="po", name="psv")
                for dt in range(ndt):
                    nc.tensor.matmul(
                        psv[:],
                        xkv_sb[dt][:, t * 128 : (t + 1) * 128],
                        wv_sb[:, dt, :],
                        start=(dt == 0),
                        stop=(dt == ndt - 1),
                    )
                nc.vector.tensor_copy(
                    v_sb[:, t, :].rearrange("p (h w) -> p h w", w=vw)[:, :, 0:d_head],
                    psv[:].rearrange("p (h w) -> p h w", w=d_head),
                )

        # ---- phase B: attention, phased per (tqh, pair) unit:
        # scores+exp+mul batch, then attnV batch, then the normalize tail.
        # Wo (pair-split) runs at tq-chunk boundaries: pair0 first (its tail
        # finished during pair1's unit), then pair1 (tail hides under pair0's
        # Wo matmuls).
        with (
            tc.tile_pool(name="ppool", bufs=2 * ntk + 12) as ppool,
            tc.tile_pool(name="ebpool", bufs=4) as ebpool,
        ):
            def emit_wo(tqh, pair):
                for ti in range(n_wot):
                    t = tqh * n_wot + ti
                    pf = psS.tile([128, d_model], F32, tag="ps", name="pf")
                    for m0 in range(0, d_model, NQ):
                        nc.tensor.matmul(
                            pf[:, m0 : m0 + NQ],
                            stack_sb[:, pair, t * 128 : (t + 1) * 128],
                            wo_sb[:, pair, m0 : m0 + NQ],
                            start=True,
                            stop=True,
                        )
                    osb = opool.tile([128, d_model], F16, tag="osb", name="osb")
                    eng = nc.vector.tensor_copy if ti % 2 == 0 else nc.scalar.copy
                    eng(osb[:], pf[:])
                    nc.sync.dma_start(out=out_d[pair, t * 128 : (t + 1) * 128, :], in_=osb[:])

            for tqh in range(n_tqh):
                c0 = tqh * CH
                for pair in range(pairs):
                    # scores^T + exp + expb-mul for both heads of the pair
                    p_ts = []
                    for t in range(ntk):
                        tr = slice(t * 128, (t + 1) * 128)
                        eb_t = ebpool.tile([128, 2, CH], F16, tag="eb", name="eb")
                        nc.sync.dma_start(
                            out=eb_t[:],
                            in_=eb_d.ap()[2 * pair : 2 * pair + 2, t * 128 : (t + 1) * 128,
                                          c0 : c0 + CH].rearrange("h p q -> p h q"),
                        )
                        pp = []
                        for hh in range(2):
                            r0 = hh * 64
                            psAB = psS.tile([128, CH], F32, tag="ps", name="ps")
                            for q0 in range(0, CH, NQ):
                                nc.tensor.matmul(
                                    psAB[:, q0 : q0 + NQ],
                                    kT_sb[r0 : r0 + 64, pair, tr],
                                    qT_sb[r0 : r0 + 64, pair, c0 + q0 : c0 + q0 + NQ],
                                    start=True,
                                    stop=True,
                                )
                            p_t = ppool.tile([128, CH], F16, tag="p", name="p_t")
                            nc.scalar.activation(
                                out=p_t[:], in_=psAB[:],
                                func=mybir.ActivationFunctionType.Exp, scale=scale,
                            )
                            nc.vector.tensor_mul(p_t[:], p_t[:], eb_t[:, hh, :])
                            pp.append(p_t)
                        p_ts.append(pp)

                    # attn @ [V|1] -> [65, NQ] per (head, 512-chunk)
                    sums_t = npool.tile([2 * nqc, NQ], F32, tag="sums", name="sums_t", bufs=2)
                    u_list = []
                    for hh in range(2):
                        h = 2 * pair + hh
                        po = [psO.tile([vw, NQ], F32, tag="po", name="po") for _ in range(nqc)]
                        for t in range(ntk):
                            for qi in range(nqc):
                                nc.tensor.matmul(
                                    po[qi][:],
                                    v_sb[:, t, h * vw : (h + 1) * vw],
                                    p_ts[t][hh][:, qi * NQ : (qi + 1) * NQ],
                                    start=(t == 0),
                                    stop=(t == ntk - 1),
                                )
                        for qi in range(nqc):
                            qg = tqh * nqc + qi  # global 512-chunk index
                            row = hh * nqc + qi
                            u_t = upool.tile([64, NQ], F16, tag="u", name="u_t")
                            nc.vector.tensor_copy(u_t[:], po[qi][0:64, :])
                            sm_t = npool.tile([1, NQ], F32, tag="sm", name="sm_t")
                            nc.vector.tensor_copy(sm_t[:], po[qi][64:65, :])
                            # DMA: compute engines can't address partition `row`
                            nc.sync.dma_start(out=sums_t[row : row + 1, :], in_=sm_t[:])
                            u_list.append((u_t, row, hh * 64, qg))

                    # normalize: one fast reciprocal per unit, gpsimd
                    # broadcast straight from the recip row, multiply on DVE
                    recip_f = npool.tile([2 * nqc, NQ], F32, tag="recipf", name="recip_f", bufs=2)
                    nc.vector.reciprocal_approx_fast(out=recip_f[:], in_=sums_t[:])
                    recip16 = npool.tile([2 * nqc, NQ], F16, tag="recip", name="recip16", bufs=2)
                    nc.vector.tensor_copy(recip16[:], recip_f[:])
                    for u_t, row, r0, qg in u_list:
                        r_t = npool.tile([1, NQ], F16, tag="r", name="r_t")
                        nc.sync.dma_start(out=r_t[:], in_=recip16[row : row + 1, :])
                        rb_t = npool.tile([64, NQ], F16, tag="rb", name="rb_t")
                        nc.gpsimd.partition_broadcast(rb_t[:], r_t[:])
                        nc.vector.tensor_mul(
                            stack_sb[r0 : r0 + 64, pair, qg * NQ : (qg + 1) * NQ],
                            u_t[:],
                            rb_t[:],
                        )
                # boundary: out-projection for this tq chunk, pair0 then pair1
                for pair in range(pairs):
                    emit_wo(tqh, pair)

    nc.compile()
    return nc


_NC = None
LAST_RESULTS = None


def _get_nc():
    global _NC
    if _NC is None:
        _NC = build_nc()
    return _NC


def _shard_inputs(query, key_value, mask, rel_pos_bias, Wq, Wkv, Wo):
    """Build the 8 per-core input maps (host-side transposes + exp-bias)."""
    in_maps = []
    w_f16 = {
        "Wq": Wq.astype(NP_F16),
        "Wo": Wo.astype(NP_F16),
        "Wkv": Wkv.astype(NP_F16),
    }
    for c in range(N_CORES):
        b = c // (N_CORES // B)
        g = c % (N_CORES // B)
        cs = slice(g * HPC * D_HEAD, (g + 1) * HPC * D_HEAD)
        hs = slice(g * HPC, (g + 1) * HPC)
        # expb = exp(bias)^T * mask^T   (fp32 exp, fp16 ship)
        eb = np.exp(rel_pos_bias[hs].astype(np.float32)).transpose(0, 2, 1)
        eb = eb * mask[b, 0].T[None].astype(np.float32)
        in_maps.append({
            "xqT": np.ascontiguousarray(query[b].T).astype(NP_F16),
            "xkvT": np.ascontiguousarray(key_value[b].T).astype(NP_F16),
            "wq": w_f16["Wq"][:, cs].copy(),
            "wk": w_f16["Wkv"][:, cs].copy(),
            "wv": w_f16["Wkv"][:, D_MODEL + cs.start : D_MODEL + cs.stop].copy(),
            "wo": w_f16["Wo"][cs, :].copy(),
            "expb": eb.astype(NP_F16),
        })
    return in_maps


def kernel(query, key_value, mask, rel_pos_bias, Wq, Wkv, Wo):
    global LAST_RESULTS
    query, key_value, mask, rel_pos_bias, Wq, Wkv, Wo = (
        np.asarray(a) for a in (query, key_value, mask, rel_pos_bias, Wq, Wkv, Wo)
    )
    nc = _get_nc()
    in_maps = _shard_inputs(query, key_value, mask, rel_pos_bias, Wq, Wkv, Wo)
    res = run_bass_kernel_spmd(nc, in_maps, core_ids=list(range(N_CORES)))
    LAST_RESULTS = res
    gpc = N_CORES // B  # cores per batch group
    out = np.stack([
        sum(res.results[b * gpc + i]["out"].astype(np.float32).sum(axis=0) for i in range(gpc))
        for b in range(B)
    ])
    return out


# revision 12
# speedup vs baseline: 1.6658x; 1.0260x over previous
"""MultiHeadCrossAttention Trainium2 Bass kernel (v3).

Sharding (8 cores): data-parallel over batch (2) x tensor-parallel over
head groups (4 groups of 4 heads).  Core c handles batch c//4, heads
4*(c%4) .. 4*(c%4)+3.  Each core computes TWO partial [Tq, D] outputs
(one per head-pair through its Wo row-slice); the host sums the 8
partials per batch.

Device math per core (all matmuls fp16 x fp16 -> fp32 PSUM):
  qT = Wq_s.T @ Xq.T          [256, Tq]   (head-dim on partitions)
  kT = Wk_s.T @ Xkv.T         [256, Tk]
  V  = Xkv @ Wv_s             [Tk, 256]   (+ ones column per head)
  St = kT_h.T @ qT_h          [Tk, Tq] scores^T, K=64, head pairs packed
                              into PE row-groups 0-63 / 64-127
  E  = exp(St/8)              (ScalarE, scale folded into activation)
  P  = E * expb               expb = exp(bias^T) * mask^T  (host-built;
                              multiplicative bias: exp(s+b) = exp(s)exp(b))
  [out^T; sums] = [V_h|1].T @ P   [65, Tq]  ones-column gives softmax sums
  out_norm^T = out^T * (1/sums)   (approx reciprocal + gpsimd broadcast)
  partial[pair] = stack_pair.T @ Wo_pair  [Tq, D] per pair (host sums)

Wo is pair-split and emitted at tq-chunk boundaries ordered pair0 then
pair1, so pair1's normalize tail hides under pair0's Wo matmuls.  The
small tail DMAs (sums gather, reciprocal row moves) ride the Vector
engine's DGE ring so they never queue behind the bulk expb prefetch
stream on the Sync ring.

Softmax max-subtraction is skipped: logits ~ N(0, ~1.1), max |logit| < ~7
over 16M samples, exp stays in fp16/fp32 range comfortably.
"""

import os
from contextlib import ExitStack

import numpy as np

import concourse.bass as bass
import concourse.mybir as mybir
import concourse.tile as tile
from concourse import bacc
from concourse.bass_utils import run_bass_kernel_spmd

# Problem dims (hardcoded per contract).
D_MODEL = 1024
NUM_HEADS = 16
D_HEAD = 64
B = 2
TQ = 2048
TK = 2048
N_CORES = 8
HPC = 4  # heads per core
SCALE = 1.0 / 8.0  # 1/sqrt(D_HEAD)

F16 = mybir.dt.float16
F32 = mybir.dt.float32
NP_F16 = np.float16

NQ = 512  # matmul moving free-dim chunk (PSUM bank = 512 fp32)


def build_nc(d_model=D_MODEL, tq=TQ, tk=TK, hpc=HPC, d_head=D_HEAD, scale=SCALE):
    """Build the single-core Bass program (SPMD: same NEFF on all cores)."""
    assert d_model % 128 == 0 and tq % NQ == 0 and tk % 128 == 0
    assert hpc % 2 == 0
    ndt = d_model // 128          # contraction tiles for projections
    pairs = hpc // 2              # head pairs (128 head-dims per pair)
    hd = hpc * d_head             # per-core head dims (= 256)
    ntq = tq // NQ                # Tq chunks of 512
    ntk = tk // 128               # Tk tiles of 128
    vw = d_head + 1               # V columns per head incl. ones column
    CH = min(tq, 1024)            # scores psum tile width (2 PSUM banks)
    nqc = CH // NQ                # 512-chunks per scores tile
    n_tqh = tq // CH              # tq macro-chunks per head
    n_wot = CH // 128             # Wo row-tiles per tq chunk (= 8)

    nc = bacc.Bacc("TRN2", target_bir_lowering=False, debug=False)

    xq_d = nc.dram_tensor("xqT", [d_model, tq], F16, kind="ExternalInput")
    xkv_d = nc.dram_tensor("xkvT", [d_model, tk], F16, kind="ExternalInput")
    wq_d = nc.dram_tensor("wq", [d_model, hd], F16, kind="ExternalInput")
    wk_d = nc.dram_tensor("wk", [d_model, hd], F16, kind="ExternalInput")
    wv_d = nc.dram_tensor("wv", [d_model, hd], F16, kind="ExternalInput")
    wo_d = nc.dram_tensor("wo", [hd, d_model], F16, kind="ExternalInput")
    eb_d = nc.dram_tensor("expb", [hpc, tk, tq], F16, kind="ExternalInput")
    out_d = nc.dram_tensor("out", [pairs, tq, d_model], F16, kind="ExternalOutput")

    with ExitStack() as ctx:
        tc = ctx.enter_context(tile.TileContext(nc))
        # ---- persistent pools
        wpool = ctx.enter_context(tc.tile_pool(name="wpool", bufs=1))
        qkpool = ctx.enter_context(tc.tile_pool(name="qkpool", bufs=1))
        opool = ctx.enter_context(tc.tile_pool(name="opool", bufs=8))
        npool = ctx.enter_context(tc.tile_pool(name="npool", bufs=4))
        upool = ctx.enter_context(tc.tile_pool(name="upool", bufs=hpc * (tq // NQ)))
        psS = ctx.enter_context(tc.tile_pool(name="psS", bufs=3, space="PSUM"))
        psO = ctx.enter_context(tc.tile_pool(name="psO", bufs=2, space="PSUM"))

        wq_sb = wpool.tile([128, ndt, hd], F16, tag="wq")
        wk_sb = wpool.tile([128, ndt, hd], F16, tag="wk")
        wv_sb = wpool.tile([128, ndt, hd], F16, tag="wv")
        wo_sb = wpool.tile([128, pairs, d_model], F16, tag="wo")
        nc.sync.dma_start(out=wk_sb[:], in_=wk_d.ap().rearrange("(t p) j -> p t j", p=128))
        nc.sync.dma_start(out=wv_sb[:], in_=wv_d.ap().rearrange("(t p) j -> p t j", p=128))

        qT_sb = qkpool.tile([128, pairs, tq], F16, tag="qT")
        kT_sb = qkpool.tile([128, pairs, tk], F16, tag="kT")
        v_sb = qkpool.tile([128, ntk, hpc * vw], F16, tag="v")
        stack_sb = qkpool.tile([128, pairs, tq], F16, tag="stack")

        # ones columns of v_sb (projection copies overwrite the V columns)
        nc.gpsimd.memset(v_sb[:], 1.0)

        # ---- phase A: projections (X^T resident only here)
        with tc.tile_pool(name="xpool", bufs=1) as xpool:
            # one tile per d-slice so each projection matmul depends only on
            # its own 0.5 MB DMA (kv first: kT, V and scores need it)
            xkv_sb = [xpool.tile([128, tk], F16, tag=f"xkv{dt}", name="xkv_sb") for dt in range(ndt)]
            xq_sb = [xpool.tile([128, tq], F16, tag=f"xq{dt}", name="xq_sb") for dt in range(ndt)]
            for dt in range(ndt):
                nc.sync.dma_start(out=xkv_sb[dt][:], in_=xkv_d[dt * 128 : (dt + 1) * 128, :])
            nc.sync.dma_start(out=wq_sb[:], in_=wq_d.ap().rearrange("(t p) j -> p t j", p=128))
            for dt in range(ndt):
                nc.sync.dma_start(out=xq_sb[dt][:], in_=xq_d[dt * 128 : (dt + 1) * 128, :])
            nc.sync.dma_start(out=wo_sb[:], in_=wo_d.ap().rearrange("(t p) m -> p t m", p=128))

            # qT / kT: [j-pair 128, tq]  = sum_d W[:, j].T @ X^T
            for wsb, xsb, dst, tlen in ((wk_sb, xkv_sb, kT_sb, tk), (wq_sb, xq_sb, qT_sb, tq)):
                for j in range(pairs):
                    for c0 in range(0, tlen, CH):
                        cn = min(CH, tlen - c0)
                        ps = psS.tile([128, cn], F32, tag="ps", name="ps")
                        for dt in range(ndt):
                            for q0 in range(0, cn, NQ):
                                qn = min(NQ, cn - q0)
                                nc.tensor.matmul(
                                    ps[:, q0 : q0 + qn],
                                    wsb[:, dt, j * 128 : (j + 1) * 128],
                                    xsb[dt][:, c0 + q0 : c0 + q0 + qn],
                                    start=(dt == 0),
                                    stop=(dt == ndt - 1),
                                )
                        nc.vector.tensor_copy(dst[:, j, c0 : c0 + cn], ps[:])

            # V: [tk 128, hd] = X_kv @ Wv ; scatter per head next to ones cols
            for t in range(ntk):
                psv = psO.tile([128, hd], F32, tag="po", name="psv")
                for dt in range(ndt):
                    nc.tensor.matmul(
                        psv[:],
                        xkv_sb[dt][:, t * 128 : (t + 1) * 128],
                        wv_sb[:, dt, :],
                        start=(dt == 0),
                        stop=(dt == ndt - 1),
                    )
                nc.vector.tensor_copy(
                    v_sb[:, t, :].rearrange("p (h w) -> p h w", w=vw)[:, :, 0:d_head],
                    psv[:].rearrange("p (h w) -> p h w", w=d_head),
                )

        # ---- phase B: attention, phased per (tqh, pair) unit:
        # scores+exp+mul batch, then attnV batch, then the normalize tail.
        # Wo (pair-split) runs at tq-chunk boundaries: pair0 first (its tail
        # finished during pair1's unit), then pair1 (tail hides under pair0's
        # Wo matmuls).
        with (
            tc.tile_pool(name="ppool", bufs=2 * ntk + 8) as ppool,
            tc.tile_pool(name="ebpool", bufs=4) as ebpool,
        ):
            def emit_wo(tqh, pair):
                for ti in range(n_wot):
                    t = tqh * n_wot + ti
                    pf = psS.tile([128, d_model], F32, tag="ps", name="pf")
                    for m0 in range(0, d_model, NQ):
                        nc.tensor.matmul(
                            pf[:, m0 : m0 + NQ],
                            stack_sb[:, pair, t * 128 : (t + 1) * 128],
                            wo_sb[:, pair, m0 : m0 + NQ],
                            start=True,
                            stop=True,
                        )
                    osb = opool.tile([128, d_model], F16, tag="osb", name="osb")
                    eng = nc.vector.tensor_copy if ti % 2 == 0 else nc.scalar.copy
                    eng(osb[:], pf[:])
                    nc.sync.dma_start(out=out_d[pair, t * 128 : (t + 1) * 128, :], in_=osb[:])

            for tqh in range(n_tqh):
                c0 = tqh * CH
                for pair in range(pairs):
                    # scores^T + exp + expb-mul for both heads of the pair
                    p_ts = []
                    for t in range(ntk):
                        tr = slice(t * 128, (t + 1) * 128)
                        eb_t = ebpool.tile([128, 2, CH], F16, tag="eb", name="eb")
                        nc.sync.dma_start(
                            out=eb_t[:],
                            in_=eb_d.ap()[2 * pair : 2 * pair + 2, t * 128 : (t + 1) * 128,
                                          c0 : c0 + CH].rearrange("h p q -> p h q"),
                        )
                        pp = []
                        for hh in range(2):
                            r0 = hh * 64
                            psAB = psS.tile([128, CH], F32, tag="ps", name="ps")
                            for q0 in range(0, CH, NQ):
                                nc.tensor.matmul(
                                    psAB[:, q0 : q0 + NQ],
                                    kT_sb[r0 : r0 + 64, pair, tr],
                                    qT_sb[r0 : r0 + 64, pair, c0 + q0 : c0 + q0 + NQ],
                                    start=True,
                                    stop=True,
                                )
                            p_t = ppool.tile([128, CH], F16, tag="p", name="p_t")
                            nc.scalar.activation(
                                out=p_t[:], in_=psAB[:],
                                func=mybir.ActivationFunctionType.Exp, scale=scale,
                            )
                            nc.vector.tensor_mul(p_t[:], p_t[:], eb_t[:, hh, :])
                            pp.append(p_t)
                        p_ts.append(pp)

                    # attn @ [V|1] -> [65, NQ] per (head, 512-chunk)
                    sums_t = npool.tile([2 * nqc, NQ], F32, tag="sums", name="sums_t", bufs=2)
                    u_list = []
                    for hh in range(2):
                        h = 2 * pair + hh
                        po = [psO.tile([vw, NQ], F32, tag="po", name="po") for _ in range(nqc)]
                        for t in range(ntk):
                            for qi in range(nqc):
                                nc.tensor.matmul(
                                    po[qi][:],
                                    v_sb[:, t, h * vw : (h + 1) * vw],
                                    p_ts[t][hh][:, qi * NQ : (qi + 1) * NQ],
                                    start=(t == 0),
                                    stop=(t == ntk - 1),
                                )
                        for qi in range(nqc):
                            row = hh * nqc + qi
                            sm_t = npool.tile([1, NQ], F32, tag="sm", name="sm_t")
                            nc.scalar.copy(sm_t[:], po[qi][64:65, :])
                            # DMA: compute engines can't address partition `row`
                            nc.sync.dma_start(out=sums_t[row : row + 1, :], in_=sm_t[:])
                        for qi in range(nqc):
                            qg = tqh * nqc + qi  # global 512-chunk index
                            row = hh * nqc + qi
                            u_t = upool.tile([64, NQ], F16, tag="u", name="u_t")
                            nc.scalar.copy(u_t[:], po[qi][0:64, :])
                            u_list.append((u_t, row, hh * 64, qg))

                    # normalize: one fast reciprocal per unit, gpsimd
                    # broadcast straight from the recip row, multiply on DVE
                    recip_f = npool.tile([2 * nqc, NQ], F32, tag="recipf", name="recip_f", bufs=2)
                    nc.vector.reciprocal_approx_fast(out=recip_f[:], in_=sums_t[:])
                    recip16 = npool.tile([2 * nqc, NQ], F16, tag="recip", name="recip16", bufs=2)
                    nc.vector.tensor_copy(recip16[:], recip_f[:])
                    for u_t, row, r0, qg in u_list:
                        r_t = npool.tile([1, NQ], F16, tag="r", name="r_t")
                        nc.sync.dma_start(out=r_t[:], in_=recip16[row : row + 1, :])
                        rb_t = npool.tile([64, NQ], F16, tag="rb", name="rb_t")
                        nc.gpsimd.partition_broadcast(rb_t[:], r_t[:])
                        nc.vector.tensor_mul(
                            stack_sb[r0 : r0 + 64, pair, qg * NQ : (qg + 1) * NQ],
                            u_t[:],
                            rb_t[:],
                        )
                # boundary: out-projection for this tq chunk, pair0 then pair1
                for pair in range(pairs):
                    emit_wo(tqh, pair)

    nc.compile()
    return nc


_NC = None
LAST_RESULTS = None


def _get_nc():
    global _NC
    if _NC is None:
        _NC = build_nc()
    return _NC


def _shard_inputs(query, key_value, mask, rel_pos_bias, Wq, Wkv, Wo):
    """Build the 8 per-core input maps (host-side transposes + exp-bias)."""
    in_maps = []
    w_f16 = {
        "Wq": Wq.astype(NP_F16),
        "Wo": Wo.astype(NP_F16),
        "Wkv": Wkv.astype(NP_F16),
    }
    for c in range(N_CORES):
        b = c // (N_CORES // B)
        g = c % (N_CORES // B)
        cs = slice(g * HPC * D_HEAD, (g + 1) * HPC * D_HEAD)
        hs = slice(g * HPC, (g + 1) * HPC)
        # expb = exp(bias)^T * mask^T   (fp32 exp, fp16 ship)
        eb = np.exp(rel_pos_bias[hs].astype(np.float32)).transpose(0, 2, 1)
        eb = eb * mask[b, 0].T[None].astype(np.float32)
        in_maps.append({
            "xqT": np.ascontiguousarray(query[b].T).astype(NP_F16),
            "xkvT": np.ascontiguousarray(key_value[b].T).astype(NP_F16),
            "wq": w_f16["Wq"][:, cs].copy(),
            "wk": w_f16["Wkv"][:, cs].copy(),
            "wv": w_f16["Wkv"][:, D_MODEL + cs.start : D_MODEL + cs.stop].copy(),
            "wo": w_f16["Wo"][cs, :].copy(),
            "expb": eb.astype(NP_F16),
        })
    return in_maps


def kernel(query, key_value, mask, rel_pos_bias, Wq, Wkv, Wo):
    global LAST_RESULTS
    query, key_value, mask, rel_pos_bias, Wq, Wkv, Wo = (
        np.asarray(a) for a in (query, key_value, mask, rel_pos_bias, Wq, Wkv, Wo)
    )
    nc = _get_nc()
    in_maps = _shard_inputs(query, key_value, mask, rel_pos_bias, Wq, Wkv, Wo)
    res = run_bass_kernel_spmd(nc, in_maps, core_ids=list(range(N_CORES)))
    LAST_RESULTS = res
    gpc = N_CORES // B  # cores per batch group
    out = np.stack([
        sum(res.results[b * gpc + i]["out"].astype(np.float32).sum(axis=0) for i in range(gpc))
        for b in range(B)
    ])
    return out


# revision 14
# speedup vs baseline: 1.7329x; 1.0403x over previous
"""MultiHeadCrossAttention Trainium2 Bass kernel (v3).

Sharding (8 cores): data-parallel over batch (2) x tensor-parallel over
head groups (4 groups of 4 heads).  Core c handles batch c//4, heads
4*(c%4) .. 4*(c%4)+3.  Each core computes TWO partial [Tq, D] outputs
(one per head-pair through its Wo row-slice); the host sums the 8
partials per batch.

Device math per core (all matmuls fp16 x fp16 -> fp32 PSUM):
  qT = Wq_s.T @ Xq.T          [256, Tq]   (head-dim on partitions)
  kT = Wk_s.T @ Xkv.T         [256, Tk]
  V  = Xkv @ Wv_s             [Tk, 256]   (+ ones column per head)
  St = kT_h.T @ qT_h          [Tk, Tq] scores^T, K=64, head pairs packed
                              into PE row-groups 0-63 / 64-127
  E  = exp(St/8)              (ScalarE, scale folded into activation)
  P  = E * expb               expb = exp(bias^T) * mask^T  (host-built;
                              multiplicative bias: exp(s+b) = exp(s)exp(b))
  [out^T; sums] = [V_h|1].T @ P   [65, Tq]  ones-column gives softmax sums
  out_norm^T = out^T * (1/sums)   (approx reciprocal + gpsimd broadcast)
  partial[pair] = stack_pair.T @ Wo_pair  [Tq, D] per pair (host sums)

Wo is pair-split and emitted at tq-chunk boundaries ordered pair0 then
pair1, so pair1's normalize tail hides under pair0's Wo matmuls.  The
small tail DMAs (sums gather, reciprocal row moves) ride the Vector
engine's DGE ring so they never queue behind the bulk expb prefetch
stream on the Sync ring.

Softmax max-subtraction is skipped: logits ~ N(0, ~1.1), max |logit| < ~7
over 16M samples, exp stays in fp16/fp32 range comfortably.
"""

import os
from contextlib import ExitStack

import numpy as np

import concourse.bass as bass
import concourse.mybir as mybir
import concourse.tile as tile
from concourse import bacc
from concourse.bass_utils import run_bass_kernel_spmd

# Problem dims (hardcoded per contract).
D_MODEL = 1024
NUM_HEADS = 16
D_HEAD = 64
B = 2
TQ = 2048
TK = 2048
N_CORES = 8
HPC = 4  # heads per core
SCALE = 1.0 / 8.0  # 1/sqrt(D_HEAD)

F16 = mybir.dt.float16
F32 = mybir.dt.float32
NP_F16 = np.float16

NQ = 512  # matmul moving free-dim chunk (PSUM bank = 512 fp32)


def build_nc(d_model=D_MODEL, tq=TQ, tk=TK, hpc=HPC, d_head=D_HEAD, scale=SCALE):
    """Build the single-core Bass program (SPMD: same NEFF on all cores)."""
    assert d_model % 128 == 0 and tq % NQ == 0 and tk % 128 == 0
    assert hpc % 2 == 0
    ndt = d_model // 128          # contraction tiles for projections
    pairs = hpc // 2              # head pairs (128 head-dims per pair)
    hd = hpc * d_head             # per-core head dims (= 256)
    ntq = tq // NQ                # Tq chunks of 512
    ntk = tk // 128               # Tk tiles of 128
    vw = d_head + 1               # V columns per head incl. ones column
    CH = min(tq, 1024)            # scores psum tile width (2 PSUM banks)
    nqc = CH // NQ                # 512-chunks per scores tile
    n_tqh = tq // CH              # tq macro-chunks per head
    n_wot = CH // 128             # Wo row-tiles per tq chunk (= 8)

    nc = bacc.Bacc("TRN2", target_bir_lowering=False, debug=False)

    xq_d = nc.dram_tensor("xqT", [d_model, tq], F16, kind="ExternalInput")
    xkv_d = nc.dram_tensor("xkvT", [d_model, tk], F16, kind="ExternalInput")
    wq_d = nc.dram_tensor("wq", [d_model, hd], F16, kind="ExternalInput")
    wk_d = nc.dram_tensor("wk", [d_model, hd], F16, kind="ExternalInput")
    wv_d = nc.dram_tensor("wv", [d_model, hd], F16, kind="ExternalInput")
    wo_d = nc.dram_tensor("wo", [hd, d_model], F16, kind="ExternalInput")
    eb_d = nc.dram_tensor("expb", [hpc, tk, tq], F16, kind="ExternalInput")
    out_d = nc.dram_tensor("out", [pairs, tq, d_model], F16, kind="ExternalOutput")

    with ExitStack() as ctx:
        tc = ctx.enter_context(tile.TileContext(nc))
        # ---- persistent pools
        wpool = ctx.enter_context(tc.tile_pool(name="wpool", bufs=1))
        qkpool = ctx.enter_context(tc.tile_pool(name="qkpool", bufs=1))
        opool = ctx.enter_context(tc.tile_pool(name="opool", bufs=8))
        npool = ctx.enter_context(tc.tile_pool(name="npool", bufs=4))
        upool = ctx.enter_context(tc.tile_pool(name="upool", bufs=hpc * (tq // NQ)))
        psS = ctx.enter_context(tc.tile_pool(name="psS", bufs=3, space="PSUM"))
        psO = ctx.enter_context(tc.tile_pool(name="psO", bufs=2, space="PSUM"))

        wq_sb = wpool.tile([128, ndt, hd], F16, tag="wq")
        wk_sb = wpool.tile([128, ndt, hd], F16, tag="wk")
        wv_sb = wpool.tile([128, ndt, hd], F16, tag="wv")
        wo_sb = wpool.tile([128, pairs, d_model], F16, tag="wo")
        nc.sync.dma_start(out=wk_sb[:], in_=wk_d.ap().rearrange("(t p) j -> p t j", p=128))
        nc.sync.dma_start(out=wv_sb[:], in_=wv_d.ap().rearrange("(t p) j -> p t j", p=128))

        qT_sb = qkpool.tile([128, pairs, tq], F16, tag="qT")
        kT_sb = qkpool.tile([128, pairs, tk], F16, tag="kT")
        v_sb = qkpool.tile([128, ntk, hpc * vw], F16, tag="v")
        stack_sb = qkpool.tile([128, pairs, tq], F16, tag="stack")

        ones_sb = qkpool.tile([65, 64], F16, tag="ones")

        # ones columns of v_sb (projection copies overwrite the V columns)
        nc.gpsimd.memset(v_sb[:], 1.0)
        nc.gpsimd.memset(ones_sb[:], 1.0)

        # ---- phase A: projections (X^T resident only here)
        with tc.tile_pool(name="xpool", bufs=1) as xpool:
            # one tile per d-slice so each projection matmul depends only on
            # its own 0.5 MB DMA (kv first: kT, V and scores need it)
            xkv_sb = [xpool.tile([128, tk], F16, tag=f"xkv{dt}", name="xkv_sb") for dt in range(ndt)]
            xq_sb = [xpool.tile([128, tq], F16, tag=f"xq{dt}", name="xq_sb") for dt in range(ndt)]
            for dt in range(ndt):
                nc.sync.dma_start(out=xkv_sb[dt][:], in_=xkv_d[dt * 128 : (dt + 1) * 128, :])
            nc.sync.dma_start(out=wq_sb[:], in_=wq_d.ap().rearrange("(t p) j -> p t j", p=128))
            for dt in range(ndt):
                nc.sync.dma_start(out=xq_sb[dt][:], in_=xq_d[dt * 128 : (dt + 1) * 128, :])
            nc.sync.dma_start(out=wo_sb[:], in_=wo_d.ap().rearrange("(t p) m -> p t m", p=128))

            # qT / kT: [j-pair 128, tq]  = sum_d W[:, j].T @ X^T
            for wsb, xsb, dst, tlen in ((wk_sb, xkv_sb, kT_sb, tk), (wq_sb, xq_sb, qT_sb, tq)):
                for j in range(pairs):
                    for c0 in range(0, tlen, CH):
                        cn = min(CH, tlen - c0)
                        ps = psS.tile([128, cn], F32, tag="ps", name="ps")
                        for dt in range(ndt):
                            for q0 in range(0, cn, NQ):
                                qn = min(NQ, cn - q0)
                                nc.tensor.matmul(
                                    ps[:, q0 : q0 + qn],
                                    wsb[:, dt, j * 128 : (j + 1) * 128],
                                    xsb[dt][:, c0 + q0 : c0 + q0 + qn],
                                    start=(dt == 0),
                                    stop=(dt == ndt - 1),
                                )
                        nc.vector.tensor_copy(dst[:, j, c0 : c0 + cn], ps[:])

            # V: [tk 128, hd] = X_kv @ Wv ; scatter per head next to ones cols
            for t in range(ntk):
                psv = psO.tile([128, hd], F32, tag="po", name="psv")
                for dt in range(ndt):
                    nc.tensor.matmul(
                        psv[:],
                        xkv_sb[dt][:, t * 128 : (t + 1) * 128],
                        wv_sb[:, dt, :],
                        start=(dt == 0),
                        stop=(dt == ndt - 1),
                    )
                nc.vector.tensor_copy(
                    v_sb[:, t, :].rearrange("p (h w) -> p h w", w=vw)[:, :, 0:d_head],
                    psv[:].rearrange("p (h w) -> p h w", w=d_head),
                )

        # ---- phase B: attention, phased per (tqh, pair) unit:
        # scores+exp+mul batch, then attnV batch, then the normalize tail.
        # Wo (pair-split) runs at tq-chunk boundaries: pair0 first (its tail
        # finished during pair1's unit), then pair1 (tail hides under pair0's
        # Wo matmuls).
        with (
            tc.tile_pool(name="ppool", bufs=2 * ntk + 8) as ppool,
            tc.tile_pool(name="ebpool", bufs=4) as ebpool,
        ):
            def emit_wo(tqh, pair):
                for ti in range(n_wot):
                    t = tqh * n_wot + ti
                    pf = psS.tile([128, d_model], F32, tag="ps", name="pf")
                    for m0 in range(0, d_model, NQ):
                        nc.tensor.matmul(
                            pf[:, m0 : m0 + NQ],
                            stack_sb[:, pair, t * 128 : (t + 1) * 128],
                            wo_sb[:, pair, m0 : m0 + NQ],
                            start=True,
                            stop=True,
                        )
                    osb = opool.tile([128, d_model], F16, tag="osb", name="osb")
                    eng = nc.vector.tensor_copy if ti % 2 == 0 else nc.scalar.copy
                    eng(osb[:], pf[:])
                    nc.sync.dma_start(out=out_d[pair, t * 128 : (t + 1) * 128, :], in_=osb[:])

            wo_queue = []
            for tqh in range(n_tqh):
                c0 = tqh * CH
                for pair in range(pairs):
                    # scores^T + exp + expb-mul for both heads of the pair
                    p_ts = []
                    for t in range(ntk):
                        tr = slice(t * 128, (t + 1) * 128)
                        eb_t = ebpool.tile([128, 2, CH], F16, tag="eb", name="eb")
                        nc.sync.dma_start(
                            out=eb_t[:],
                            in_=eb_d.ap()[2 * pair : 2 * pair + 2, t * 128 : (t + 1) * 128,
                                          c0 : c0 + CH].rearrange("h p q -> p h q"),
                        )
                        pp = []
                        for hh in range(2):
                            r0 = hh * 64
                            psAB = psS.tile([128, CH], F32, tag="ps", name="ps")
                            for q0 in range(0, CH, NQ):
                                nc.tensor.matmul(
                                    psAB[:, q0 : q0 + NQ],
                                    kT_sb[r0 : r0 + 64, pair, tr],
                                    qT_sb[r0 : r0 + 64, pair, c0 + q0 : c0 + q0 + NQ],
                                    start=True,
                                    stop=True,
                                )
                            p_t = ppool.tile([128, CH], F16, tag="p", name="p_t")
                            nc.scalar.activation(
                                out=p_t[:], in_=psAB[:],
                                func=mybir.ActivationFunctionType.Exp, scale=scale,
                            )
                            nc.vector.tensor_mul(p_t[:], p_t[:], eb_t[:, hh, :])
                            pp.append(p_t)
                        p_ts.append(pp)

                    # out-projections whose tails completed long ago fill
                    # the gap between this sc-batch and the av-batch
                    for wt, wp in wo_queue:
                        emit_wo(wt, wp)
                    wo_queue = []

                    # attn @ [V|1] -> [65, NQ] per (head, 512-chunk)
                    # softmax sums land on partitions {0,32,64} of sums_a plus
                    # partition 0 of sums_b (SBUF AP bases must be 0/32/64)
                    sums_a = npool.tile([65, NQ], F32, tag="sums", name="sums_a", bufs=2)
                    sums_b = npool.tile([1, NQ], F32, tag="sumsb", name="sums_b", bufs=2)
                    u_list = []
                    for hh in range(2):
                        h = 2 * pair + hh
                        po = [psO.tile([vw, NQ], F32, tag="po", name="po") for _ in range(nqc)]
                        for t in range(ntk):
                            for qi in range(nqc):
                                nc.tensor.matmul(
                                    po[qi][:],
                                    v_sb[:, t, h * vw : (h + 1) * vw],
                                    p_ts[t][hh][:, qi * NQ : (qi + 1) * NQ],
                                    start=(t == 0),
                                    stop=(t == ntk - 1),
                                )
                        for qi in range(nqc):
                            row = 32 * (hh * nqc + qi)
                            dst = sums_a[row : row + 1, :] if row < 96 else sums_b[:]
                            nc.scalar.copy(dst, po[qi][64:65, :])
                        for qi in range(nqc):
                            qg = tqh * nqc + qi  # global 512-chunk index
                            row = 32 * (hh * nqc + qi)
                            u_t = upool.tile([64, NQ], F16, tag="u", name="u_t")
                            nc.scalar.copy(u_t[:], po[qi][0:64, :])
                            u_list.append((u_t, row, hh * 64, qg))

                    # normalize: one fast reciprocal per unit, gpsimd
                    # broadcast straight from the recip row, multiply on DVE
                    recip_fa = npool.tile([65, NQ], F32, tag="recipfa", name="recip_fa", bufs=2)
                    nc.vector.reciprocal_approx_fast(out=recip_fa[:], in_=sums_a[:])
                    recip_fb = npool.tile([1, NQ], F32, tag="recipfb", name="recip_fb", bufs=2)
                    nc.vector.reciprocal_approx_fast(out=recip_fb[:], in_=sums_b[:])
                    recip_a = npool.tile([65, NQ], F16, tag="recipa", name="recip_a", bufs=2)
                    nc.vector.tensor_copy(recip_a[:], recip_fa[:])
                    recip_b = npool.tile([1, NQ], F16, tag="recipb", name="recip_b", bufs=2)
                    nc.vector.tensor_copy(recip_b[:], recip_fb[:])
                    for u_t, row, r0, qg in u_list:
                        # broadcast 1/sums across partitions via a K=1 matmul
                        # (PE tile rows 0/32/64; 4th chunk rides sums_b row 0)
                        if row < 96:
                            lhs, rhs = ones_sb[row : row + 1, :], recip_a[row : row + 1, :]
                        else:
                            lhs, rhs = ones_sb[0:1, :], recip_b[:]
                        rb_ps = psO.tile([64, NQ], F32, tag="po", name="rb_ps")
                        nc.tensor.matmul(rb_ps[:], lhs, rhs, start=True, stop=True)
                        nc.vector.tensor_mul(
                            stack_sb[r0 : r0 + 64, pair, qg * NQ : (qg + 1) * NQ],
                            u_t[:],
                            rb_ps[:],
                        )
                    wo_queue.append((tqh, pair))
            # drain remaining out-projections (last chunk's pairs)
            for wt, wp in wo_queue:
                emit_wo(wt, wp)

    nc.compile()
    return nc


_NC = None
LAST_RESULTS = None


def _get_nc():
    global _NC
    if _NC is None:
        _NC = build_nc()
    return _NC


def _shard_inputs(query, key_value, mask, rel_pos_bias, Wq, Wkv, Wo):
    """Build the 8 per-core input maps (host-side transposes + exp-bias)."""
    in_maps = []
    w_f16 = {
        "Wq": Wq.astype(NP_F16),
        "Wo": Wo.astype(NP_F16),
        "Wkv": Wkv.astype(NP_F16),
    }
    for c in range(N_CORES):
        b = c // (N_CORES // B)
        g = c % (N_CORES // B)
        cs = slice(g * HPC * D_HEAD, (g + 1) * HPC * D_HEAD)
        hs = slice(g * HPC, (g + 1) * HPC)
        # expb = exp(bias)^T * mask^T   (fp32 exp, fp16 ship)
        eb = np.exp(rel_pos_bias[hs].astype(np.float32)).transpose(0, 2, 1)
        eb = eb * mask[b, 0].T[None].astype(np.float32)
        in_maps.append({
            "xqT": np.ascontiguousarray(query[b].T).astype(NP_F16),
            "xkvT": np.ascontiguousarray(key_value[b].T).astype(NP_F16),
            "wq": w_f16["Wq"][:, cs].copy(),
            "wk": w_f16["Wkv"][:, cs].copy(),
            "wv": w_f16["Wkv"][:, D_MODEL + cs.start : D_MODEL + cs.stop].copy(),
            "wo": w_f16["Wo"][cs, :].copy(),
            "expb": eb.astype(NP_F16),
        })
    return in_maps


def kernel(query, key_value, mask, rel_pos_bias, Wq, Wkv, Wo):
    global LAST_RESULTS
    query, key_value, mask, rel_pos_bias, Wq, Wkv, Wo = (
        np.asarray(a) for a in (query, key_value, mask, rel_pos_bias, Wq, Wkv, Wo)
    )
    nc = _get_nc()
    in_maps = _shard_inputs(query, key_value, mask, rel_pos_bias, Wq, Wkv, Wo)
    res = run_bass_kernel_spmd(nc, in_maps, core_ids=list(range(N_CORES)))
    LAST_RESULTS = res
    gpc = N_CORES // B  # cores per batch group
    out = np.stack([
        sum(res.results[b * gpc + i]["out"].astype(np.float32).sum(axis=0) for i in range(gpc))
        for b in range(B)
    ])
    return out
